# revision 1
# baseline (speedup 1.0000x reference)
"""Trainium2 Bass kernel v2 for nn_CustomMultiheadAttention (linear attention
with low-rank QKV projections), SPMD over 8 NeuronCores.

Sharding: (batch, seq-half) -> core; pairwise AllReduce of kv between the two
cores sharing a batch.

v2 changes vs baseline:
- Host-side transposes: x shipped feature-major [E, T]; weights shipped as
  combined (Wu @ Wd) in device tile layout -> no PE transposes, no PSUM->SBUF
  transpose copies.
- Single-stage projections with the combined weight (same FLOPs as down+up).
- kv accumulated in PSUM across all chunks (start/stop flags).
- Denominator: block-diag ksum matmul -> [16, CH] denoms, one reciprocal,
  PE-matmul broadcast, single scale of q before the num matmul.
- AllReduce overlapped with the q feature-map pass.
- Element-wise work spread across engines via nc.any.
"""

import contextlib

import numpy as np

import concourse.bass as bass
import concourse.tile as tile
from concourse import bacc, mybir
from concourse.bass_utils import run_bass_kernel_spmd

F32 = mybir.dt.float32
F32R = mybir.dt.float32r
BF16 = mybir.dt.bfloat16

B, S, E, H, R = 4, 4096, 1024, 16, 512
D = E // H  # 64
N_CORES = 8


def build_nc(T, n_cores, groups):
    CH = min(512, T)        # tokens per chunk
    NCH = T // CH           # chunks
    TB = CH // 128          # 128-token blocks per chunk
    EC = E // 128           # 8
    FC5 = E // 512          # 2
    HP = H // 2             # 8 head pairs

    nc = bacc.Bacc("TRN2", target_bir_lowering=False, debug=False,
                   num_devices=n_cores)

    xqT = nc.declare_dram_parameter("xqT", [E, T], BF16, isOutput=False).ap()
    xkT = nc.declare_dram_parameter("xkT", [E, T], BF16, isOutput=False).ap()
    xvT = nc.declare_dram_parameter("xvT", [E, T], BF16, isOutput=False).ap()
    # combined (up@down) projection weights, tile layout [128, EC(in), E(out)]
    wqc = nc.declare_dram_parameter("wqc", [128, EC, E], BF16, isOutput=False).ap()
    wkc = nc.declare_dram_parameter("wkc", [128, EC, E], BF16, isOutput=False).ap()
    wvc = nc.declare_dram_parameter("wvc", [128, EC, E], BF16, isOutput=False).ap()
    owT = nc.declare_dram_parameter("owT", [128, EC, E], F32R, isOutput=False).ap()
    # biases: qu_b feature-major per-partition cols; others row-broadcast
    qu_bc_d = nc.declare_dram_parameter("qu_bc", [128, EC], F32, isOutput=False).ap()
    ku_bc_d = nc.declare_dram_parameter("ku_bc", [128, E], F32, isOutput=False).ap()
    vu_bc_d = nc.declare_dram_parameter("vu_bc", [128, E], F32, isOutput=False).ap()
    ou_bc_d = nc.declare_dram_parameter("ou_bc", [128, E], F32, isOutput=False).ap()
    ind_d = nc.declare_dram_parameter("ind", [16, EC, 128], F32R,
                                      isOutput=False).ap()
    y = nc.declare_dram_parameter("y", [T, E], F32, isOutput=True).ap()

    cc_in = nc.dram_tensor("cc_in", [128, HP, D + 1], F32).ap()
    cc_out = nc.dram_tensor("cc_out", [128, HP, D + 1], F32).ap()

    Exp = mybir.ActivationFunctionType.Exp
    Relu = mybir.ActivationFunctionType.Relu
    Copy = mybir.ActivationFunctionType.Copy

    with tile.TileContext(nc) as tc:
        with (
            tc.tile_pool(name="const", bufs=1) as const,
            tc.tile_pool(name="persist", bufs=1) as persist,
        ):
            kubc = const.tile([128, E], F32)
            vubc = const.tile([128, E], F32)
            qubc = const.tile([128, EC], F32)
            nc.sync.dma_start(out=kubc[:], in_=ku_bc_d[:, :])
            nc.sync.dma_start(out=vubc[:], in_=vu_bc_d[:, :])
            nc.sync.dma_start(out=qubc[:], in_=qu_bc_d[:, :])

            qfm_all = persist.tile([128, EC, T], BF16)  # elu(q)+1, feat-major
            kv_acc = persist.tile([128, HP, D + 1], F32)  # kv|ksum accum

            # Q-phase pools opened early so wqT + xq chunk 0 can prefetch
            # during the KV phase
            _stack = contextlib.ExitStack()
            wqp = _stack.enter_context(tc.tile_pool(name="wq", bufs=1))
            wqT = wqp.tile([128, EC, E], BF16, tag="wq")
            xqp = _stack.enter_context(tc.tile_pool(name="xq", bufs=2))
            xq_re = xqT.rearrange("(ec p) t -> p ec t", p=128)
            xq_t0 = xqp.tile([128, EC, CH], BF16, tag="xq")

            # ---------------- Phase KV ----------------
            with (
                tc.tile_pool(name="wkv", bufs=1) as wkvp,
                tc.tile_pool(name="xkv", bufs=2) as xkvp,
                tc.tile_pool(name="uppsum", bufs=2, space="PSUM") as ups,
                tc.tile_pool(name="ktv", bufs=2) as ktvp,
                tc.tile_pool(name="elu", bufs=2) as elup,
                tc.tile_pool(name="kvacc", bufs=2, space="PSUM") as kvap,
            ):
                hpf = 512 // D  # heads per 512-feature group = 8
                xk_re = xkT.rearrange("(ec p) t -> p ec t", p=128)
                xv_re = xvT.rearrange("(ec p) t -> p ec t", p=128)

                # first x tiles ahead of the weights in the DMA queue
                xk_t0 = xkvp.tile([128, EC, 128], BF16, tag="xk")
                nc.sync.dma_start(out=xk_t0[:], in_=xk_re[:, :, 0:128])
                xv_t0 = xkvp.tile([128, EC, 128], BF16, tag="xv")
                nc.sync.dma_start(out=xv_t0[:], in_=xv_re[:, :, 0:128])

                wkT = wkvp.tile([128, EC, E], BF16, tag="wk")
                wvT = wkvp.tile([128, EC, E], BF16, tag="wv")
                # per-ec pieces so the first accumulation group can start
                # as soon as its slice lands
                for ec in range(EC):
                    nc.sync.dma_start(out=wkT[:, ec, 0:512],
                                      in_=wkc[:, ec, 0:512])
                nc.sync.dma_start(out=wvT[:, :, 0:512], in_=wvc[:, :, 0:512])
                nc.sync.dma_start(out=wkT[:, :, 512:E], in_=wkc[:, :, 512:E])
                nc.sync.dma_start(out=wvT[:, :, 512:E], in_=wvc[:, :, 512:E])

                for c in range(NCH):
                    if c == min(1, NCH - 1):
                        # prefetch the q weight once startup traffic is done
                        nc.sync.dma_start(out=wqT[:], in_=wqc[:, :, :])
                    if c == min(2, NCH - 1):
                        nc.sync.dma_start(out=xq_t0[:],
                                          in_=xq_re[:, :, 0:CH])
                    ktm = ktvp.tile([128, TB, H, D + 1], BF16, tag="ktm")
                    vtm = ktvp.tile([128, TB, H, D + 1], BF16, tag="vtm")
                    nc.vector.memset(vtm[:, :, :, D:D + 1], 1.0)

                    for tb in range(TB):
                        t0 = c * CH + tb * 128
                        if c == 0 and tb == 0:
                            xk_t, xv_t = xk_t0, xv_t0
                        else:
                            xk_t = xkvp.tile([128, EC, 128], BF16, tag="xk")
                            nc.sync.dma_start(out=xk_t[:],
                                              in_=xk_re[:, :, t0:t0 + 128])
                            xv_t = xkvp.tile([128, EC, 128], BF16, tag="xv")
                            nc.sync.dma_start(out=xv_t[:],
                                              in_=xv_re[:, :, t0:t0 + 128])

                        for fc in range(FC5):
                            # K: token-major combined proj + bias + elu+1
                            ps = ups.tile([128, 512], F32, tag="ups")
                            for ec in range(EC):
                                nc.tensor.matmul(
                                    ps[:],
                                    xk_t[:, ec, :],
                                    wkT[:, ec, fc * 512:(fc + 1) * 512],
                                    start=(ec == 0), stop=(ec == EC - 1))
                            bsl = kubc[:, fc * 512:(fc + 1) * 512]
                            t = elup.tile([128, 512], F32, tag="elu_t")
                            nc.vector.tensor_add(t[:], ps[:], bsl)
                            m = elup.tile([128, 512], F32, tag="elu_m")
                            nc.gpsimd.tensor_scalar_min(m[:], t[:], 0.0)
                            e = elup.tile([128, 512], F32, tag="elu_e")
                            nc.scalar.activation(e[:], m[:], Exp)
                            r = elup.tile([128, 512], F32, tag="elu_r")
                            nc.scalar.activation(r[:], t[:], Relu)
                            dsl = ktm[:, tb, fc * hpf:(fc + 1) * hpf, 0:D]
                            nc.gpsimd.tensor_add(
                                dsl,
                                e[:].rearrange("p (h f) -> p h f", h=hpf),
                                r[:].rearrange("p (h f) -> p h f", h=hpf))

                            # V: token-major combined proj + bias
                            psv = ups.tile([128, 512], F32, tag="ups")
                            for ec in range(EC):
                                nc.tensor.matmul(
                                    psv[:],
                                    xv_t[:, ec, :],
                                    wvT[:, ec, fc * 512:(fc + 1) * 512],
                                    start=(ec == 0), stop=(ec == EC - 1))
                            vsl = vtm[:, tb, fc * hpf:(fc + 1) * hpf, 0:D]
                            bslv = vubc[:, fc * 512:(fc + 1) * 512]
                            nc.vector.tensor_add(
                                vsl,
                                psv[:].rearrange("p (h f) -> p h f", h=hpf),
                                bslv.rearrange("p (h f) -> p h f", h=hpf))

                    # kv partial for this chunk, accumulate into kv_acc
                    for h in range(H):
                        pkv = kvap.tile([64, D + 1], F32, tag="pkv")
                        for tb in range(TB):
                            nc.tensor.matmul(
                                pkv[:], ktm[:, tb, h, 0:D],
                                vtm[:, tb, h, 0:D + 1],
                                start=(tb == 0), stop=(tb == TB - 1))
                        b0 = 64 * (h % 2)
                        acc_sl = kv_acc[b0:b0 + 64, h // 2, :]
                        if c == 0:
                            nc.vector.tensor_copy(acc_sl, pkv[:])
                        else:
                            nc.vector.tensor_add(acc_sl, acc_sl, pkv[:])

            # ---------------- AllReduce (overlapped with Q pass) ----------
            nc.sync.dma_start(out=cc_in[:], in_=kv_acc[:])
            nc.gpsimd.collective_compute(
                "AllReduce", mybir.AluOpType.add,
                ins=[cc_in[:]], outs=[cc_out[:]],
                replica_groups=groups)

            # ------- phase-2 constants: start right after the collective ---
            w2p = _stack.enter_context(tc.tile_pool(name="w2", bufs=1))
            kvxp = _stack.enter_context(tc.tile_pool(name="kvx", bufs=1))

            # indicator tiles for the denominator broadcast matmul
            ind = kvxp.tile([16, EC, 128], F32R, tag="ind")
            nc.sync.dma_start(out=ind[:], in_=ind_d[:, :, :])

            kv_red = kvxp.tile([128, HP, D + 1], F32, tag="kvred")
            nc.sync.dma_start(out=kv_red[:], in_=cc_out[:])

            # block-diag kv tiles [128, 128] per head pair + ksum tiles
            kvbd = kvxp.tile([128, EC, 128], F32R, tag="kvbd")
            nc.vector.memset(kvbd[:].bitcast(F32), 0.0)
            ksb = kvxp.tile([128, EC, 16], BF16, tag="ksb")
            nc.vector.memset(ksb[:], 0.0)
            for ec in range(EC):
                nc.gpsimd.tensor_copy(kvbd[0:64, ec, 0:64],
                                      kv_red[0:64, ec, 0:D])
                nc.gpsimd.tensor_copy(kvbd[64:128, ec, 64:128],
                                      kv_red[64:128, ec, 0:D])
                nc.gpsimd.tensor_copy(ksb[0:64, ec, 2 * ec:2 * ec + 1],
                                      kv_red[0:64, ec, D:D + 1])
                nc.gpsimd.tensor_copy(
                    ksb[64:128, ec, 2 * ec + 1:2 * ec + 2],
                    kv_red[64:128, ec, D:D + 1])

            owt = w2p.tile([128, EC, E], F32R, tag="ow")
            oubc = w2p.tile([128, E], F32, tag="oubc")

            # ---------------- Q feature-map pass --------------------------
            with (
                tc.tile_pool(name="qpsum", bufs=3, space="PSUM") as qps,
                tc.tile_pool(name="elu2", bufs=2) as elu2,
            ):
                for c in range(NCH):
                    if c == min(1, NCH - 1):
                        # prefetch phase-2 weights during the q pass
                        nc.sync.dma_start(out=owt[:], in_=owT[:, :, :])
                        nc.sync.dma_start(out=oubc[:], in_=ou_bc_d[:, :])
                    if c == 0:
                        xq_t = xq_t0
                    else:
                        xq_t = xqp.tile([128, EC, CH], BF16, tag="xq")
                        nc.sync.dma_start(
                            out=xq_t[:],
                            in_=xq_re[:, :, c * CH:(c + 1) * CH])
                    for fo in range(EC):
                        ps = qps.tile([128, CH], F32, tag="qps")
                        for ec in range(EC):
                            nc.tensor.matmul(
                                ps[:],
                                wqT[:, ec, fo * 128:(fo + 1) * 128],
                                xq_t[:, ec, :],
                                start=(ec == 0), stop=(ec == EC - 1))
                        bp = qubc[:, fo:fo + 1]
                        m = elu2.tile([128, CH], F32, tag="m2")
                        nc.vector.tensor_scalar(
                            m[:], ps[:], bp, 0.0,
                            op0=mybir.AluOpType.add, op1=mybir.AluOpType.min)
                        e = elu2.tile([128, CH], F32, tag="e2")
                        nc.scalar.activation(e[:], m[:], Exp)
                        r = elu2.tile([128, CH], F32, tag="r2")
                        nc.scalar.activation(r[:], ps[:], Relu, bias=bp)
                        nc.vector.tensor_add(
                            qfm_all[:, fo, c * CH:(c + 1) * CH], e[:], r[:])

            # ---------------- Phase 2: attention + out-proj ---------------
            with (
                tc.tile_pool(name="dps", bufs=2, space="PSUM") as dps,
                tc.tile_pool(name="bcps", bufs=2, space="PSUM") as bcps,
                tc.tile_pool(name="attps", bufs=2, space="PSUM") as attps,
                tc.tile_pool(name="qsp", bufs=3) as qsp,
                tc.tile_pool(name="attp", bufs=1) as attp,
                tc.tile_pool(name="rcp", bufs=2) as rcp,
                tc.tile_pool(name="ypsum", bufs=2, space="PSUM") as yps,
                tc.tile_pool(name="ysb", bufs=2) as ysbp,
            ):
                for c in range(NCH):
                    csl = bass.ds(c * CH, CH)
                    # denominators for all 16 heads: [16, CH]
                    dn_ps = dps.tile([16, CH], F32, tag="dn")
                    for ec in range(EC):
                        nc.tensor.matmul(
                            dn_ps[:], ksb[:, ec, :], qfm_all[:, ec, csl],
                            start=(ec == 0), stop=(ec == EC - 1))
                    dn_sb = rcp.tile([16, CH], F32, tag="dnsb")
                    nc.scalar.activation(dn_sb[:], dn_ps[:], Copy, bias=1e-6)
                    rec = rcp.tile([16, CH], F32R, tag="rec")
                    with nc.allow_low_precision(reason="f32r == f32 bits"):
                        nc.vector.reciprocal(rec[:], dn_sb[:])

                    att = attp.tile([128, EC, CH], F32R, tag="att")
                    for ec in range(EC):
                        # broadcast 1/denom of the 2 heads to 128 partitions
                        bc = bcps.tile([128, CH], F32, tag="bc")
                        nc.tensor.matmul(bc[:], ind[:, ec, :], rec[:],
                                         start=True, stop=True)
                        qs = qsp.tile([128, CH], F32R, tag="qs")
                        nc.vector.tensor_mul(qs[:], qfm_all[:, ec, csl], bc[:])
                        aps = attps.tile([128, CH], F32, tag="aps")
                        nc.tensor.matmul(aps[:], kvbd[:, ec, :], qs[:],
                                         start=True, stop=True)
                        if ec % 2 == 0:
                            nc.vector.tensor_copy(att[:, ec, :], aps[:])
                        else:
                            nc.scalar.activation(att[:, ec, :], aps[:], Copy)

                    # out-projection + bias
                    for tb in range(TB):
                        ysb = ysbp.tile([128, E], F32, tag="ysb")
                        for fo in range(FC5):
                            py = yps.tile([128, 512], F32, tag="yps")
                            for ec in range(EC):
                                nc.tensor.matmul(
                                    py[:],
                                    att[:, ec, tb * 128:(tb + 1) * 128],
                                    owt[:, ec, fo * 512:(fo + 1) * 512],
                                    start=(ec == 0), stop=(ec == EC - 1))
                            nc.vector.tensor_add(
                                ysb[:, fo * 512:(fo + 1) * 512], py[:],
                                oubc[:, fo * 512:(fo + 1) * 512])
                        r0 = c * CH + tb * 128
                        nc.sync.dma_start(out=y[r0:r0 + 128, :], in_=ysb[:])

            _stack.close()

    nc.compile()
    return nc


_NC_CACHE = {}


def _get_nc(T, n_cores, groups):
    key = (T, n_cores, tuple(tuple(g) for g in groups))
    if key not in _NC_CACHE:
        _NC_CACHE[key] = build_nc(T, n_cores, groups)
    return _NC_CACHE[key]


def _tileize_in(w):
    # [A, B] -> [128, A//128, B] with out[p, a, b] = w[a*128+p, b]
    A, Bd = w.shape
    return np.ascontiguousarray(
        w.reshape(A // 128, 128, Bd).transpose(1, 0, 2))


def make_in_maps(inputs):
    """Host-side preprocessing: returns the per-core input maps."""
    query = np.asarray(inputs["query"], dtype=np.float32)
    key = np.asarray(inputs["key"], dtype=np.float32)
    value = np.asarray(inputs["value"], dtype=np.float32)
    b, s, e = query.shape
    assert (b, s, e) == (B, S, E)

    f32 = np.float32
    qd_w, qu_w, qu_b = (np.asarray(inputs[n], f32) for n in
                        ("qd_w", "qu_w", "qu_b"))
    kd_w, ku_w, ku_b = (np.asarray(inputs[n], f32) for n in
                        ("kd_w", "ku_w", "ku_b"))
    vd_w, vu_w, vu_b = (np.asarray(inputs[n], f32) for n in
                        ("vd_w", "vu_w", "vu_b"))
    out_w, out_b = (np.asarray(inputs[n], f32) for n in ("out_w", "out_b"))

    # combined projection weights: q = x @ (Wu Wd)^T + b
    # device wants wc[p, ec, f] = (Wu Wd)[f, ec*128+p] i.e. tileize of
    # (Wu Wd)^T = Wd^T Wu^T
    import ml_dtypes
    bf16 = ml_dtypes.bfloat16
    wqc = _tileize_in(qd_w.T @ qu_w.T).astype(bf16)
    wkc = _tileize_in(kd_w.T @ ku_w.T).astype(bf16)
    wvc = _tileize_in(vd_w.T @ vu_w.T).astype(bf16)
    owt = _tileize_in(out_w.T)   # owt[p, ec, fo] = out_w[fo, ec*128+p]

    qu_bc = np.ascontiguousarray(qu_b.reshape(E // 128, 128).T)
    ku_bc = np.ascontiguousarray(np.broadcast_to(ku_b, (128, E)))
    vu_bc = np.ascontiguousarray(np.broadcast_to(vu_b, (128, E)))
    ou_bc = np.ascontiguousarray(np.broadcast_to(out_b, (128, E)))

    EC = E // 128
    ind = np.zeros((16, EC, 128), dtype=np.float32)
    for ec in range(EC):
        ind[2 * ec, ec, 0:64] = 1.0
        ind[2 * ec + 1, ec, 64:128] = 1.0

    half = S // 2

    shared = {"wqc": wqc, "wkc": wkc, "wvc": wvc, "owT": owt,
              "qu_bc": qu_bc, "ku_bc": ku_bc, "vu_bc": vu_bc,
              "ou_bc": ou_bc, "ind": ind}

    in_maps = []
    for c in range(N_CORES):
        bi, hi = c // 2, c % 2
        sl = slice(hi * half, (hi + 1) * half)
        m = {
            "xqT": np.ascontiguousarray(query[bi, sl].T).astype(bf16),
            "xkT": np.ascontiguousarray(key[bi, sl].T).astype(bf16),
            "xvT": np.ascontiguousarray(value[bi, sl].T).astype(bf16),
        }
        m.update(shared)
        in_maps.append(m)

    return in_maps


def kernel(**inputs):
    in_maps = make_in_maps(inputs)
    groups = [[0, 1], [2, 3], [4, 5], [6, 7]]
    nc = _get_nc(B * S // N_CORES, N_CORES, groups)
    res = run_bass_kernel_spmd(nc, in_maps, list(range(N_CORES)))

    half = S // 2
    out = np.empty((B, S, E), dtype=np.float32)
    for c in range(N_CORES):
        bi, hi = c // 2, c % 2
        out[bi, hi * half:(hi + 1) * half] = res.results[c]["y"]
    return out

